# revision 4
# baseline (speedup 1.0000x reference)
"""Causal multi-head attention (B=1, S=4096, D=512, 8 heads x 64) on 8
Trainium2 NeuronCores. Sharding: one head per core (tensor-parallel over
n_head). Each core computes its head's QKV projections, causal attention,
and its partial output projection [4096, 512]; the host sums the 8
partials and adds the bias (the unshard step for head sharding).

Layouts (per core, f32 everywhere):
  qT/kT/vT  [512, 4096]  (host-pretransposed; contraction dim on partitions)
  wq/wk/wv  [512, 64]    (Wx[h].T - lhsT layout)
  wf        [64, 512]    (Wf[:, h].T - rhs layout)
  qhT/khT   [128, 512]x8 (rows 0:64 = head dims, rows 64:128 = duplicate for
                          PE row-packing of score matmuls)
  vh_all    [128, 65*32] (value chunks [j,dv] + ones column for row-sums)
  scores    S^T tiles [128 j, 512 i] in PSUM, exp'd on ScalarE -> SBUF
  O^T       [65, 512] PSUM accumulator per 512-query block (row 64 = rowsums)
"""

import numpy as np

import concourse.bass as bass
import concourse.tile as tile
from concourse import mybir
from concourse.bass_utils import run_bass_kernel_spmd

N_CORES = 8
DIM = 512
D_K = 64
BLK = 512  # i/s-block size
JC = 128  # j-chunk size
F32 = mybir.dt.float32


def _split_multi_waits(nc):
    """This container's walrus accepts only ONE sync-wait per instruction;
    Tile emits several. Hoist extras onto same-engine NoOps placed before
    the offending instruction."""
    n = 0
    for f in nc.m.functions:
        for bb in f.blocks:
            insts = bb.instructions
            i = 0
            while i < len(insts):
                inst = insts[i]
                si = inst.sync_info
                if si is not None and si.on_wait and len(si.on_wait) > 1:
                    waits = list(si.on_wait)
                    si.on_wait = [waits[-1]]
                    for k, w in enumerate(waits[:-1]):
                        nop = mybir.InstNoOp(
                            name=f"{inst.name}-wsplit{k}",
                            engine=inst.engine,
                            ins=[],
                            outs=[],
                            sync_info=mybir.SyncInfo(on_wait=[w], on_update=[]),
                        )
                        insts.insert(i, nop)
                        i += 1
                        n += 1
                i += 1
    return n


def build_kernel(nc, NB):
    """Emit the per-core kernel. NB = number of 512-wide seq blocks."""
    S = NB * BLK
    NCH = DIM // 128  # contraction chunks for projections
    Exp = mybir.ActivationFunctionType.Exp

    qT = nc.dram_tensor("qT", [DIM, S], F32, kind="ExternalInput").ap()
    kT = nc.dram_tensor("kT", [DIM, S], F32, kind="ExternalInput").ap()
    vT = nc.dram_tensor("vT", [DIM, S], F32, kind="ExternalInput").ap()
    wq = nc.dram_tensor("wq", [DIM, D_K], F32, kind="ExternalInput").ap()
    wk = nc.dram_tensor("wk", [DIM, D_K], F32, kind="ExternalInput").ap()
    wv = nc.dram_tensor("wv", [DIM, D_K], F32, kind="ExternalInput").ap()
    wf = nc.dram_tensor("wf", [D_K, DIM], F32, kind="ExternalInput").ap()
    tri = nc.dram_tensor("tri", [128, 128], F32, kind="ExternalInput").ap()
    ident = nc.dram_tensor("ident", [D_K, D_K], F32, kind="ExternalInput").ap()
    out = nc.dram_tensor("out", [S, DIM], F32, kind="ExternalOutput").ap()

    with tile.TileContext(nc) as tc:
        with (
            tc.tile_pool(name="persist", bufs=1) as persist,
            tc.tile_pool(name="xin", bufs=2) as xin,
            tc.tile_pool(name="pexp", bufs=3) as pexp,
            tc.tile_pool(name="osb", bufs=2) as osbp,
            tc.tile_pool(name="psP", bufs=2, space="PSUM") as psP,
            tc.tile_pool(name="psS", bufs=2, space="PSUM") as psS,
            tc.tile_pool(name="psO", bufs=2, space="PSUM") as psO,
        ):
            # --- constants & weights ---
            wq_sb = persist.tile([128, NCH * D_K], F32, tag="wq")
            wk_sb = persist.tile([128, NCH * D_K], F32, tag="wk")
            wv_sb = persist.tile([128, NCH * D_K], F32, tag="wv")
            for w_sb, w_ap in ((wq_sb, wq), (wk_sb, wk), (wv_sb, wv)):
                nc.sync.dma_start(
                    out=w_sb[:].rearrange("p (c d) -> p c d", c=NCH),
                    in_=w_ap.rearrange("(c p) d -> p c d", p=128),
                )
            wf_sb = persist.tile([D_K, DIM], F32, tag="wf")
            nc.sync.dma_start(out=wf_sb[:], in_=wf[:])
            tri_sb = persist.tile([128, 128], F32, tag="tri")
            nc.sync.dma_start(out=tri_sb[:], in_=tri[:])
            id_sb = persist.tile([D_K, D_K], F32, tag="ident")
            nc.sync.dma_start(out=id_sb[:], in_=ident[:])

            # persistent per-block tensors
            qhT = [persist.tile([128, BLK], F32, tag=f"qhT{b}", name=f"qhT{b}") for b in range(NB)]
            khT = [persist.tile([128, BLK], F32, tag=f"khT{b}", name=f"khT{b}") for b in range(NB)]
            vhT = [persist.tile([D_K, BLK], F32, tag=f"vhT{b}", name=f"vhT{b}") for b in range(NB)]
            OT = [persist.tile([D_K, BLK], F32, tag=f"OT{b}", name=f"OT{b}") for b in range(NB)]
            rsrow = [persist.tile([1, BLK], F32, tag=f"rs{b}", name=f"rs{b}") for b in range(NB)]
            njc_tot = NB * (BLK // JC)
            vh_all = persist.tile([128, 65 * njc_tot], F32, tag="vh")
            nc.gpsimd.memset(vh_all[:], 1.0)

            def proj(dst, w_sb, x_ap, sp, tag, dup):
                """Project two 512-blocks (2*sp, 2*sp+1). dst: list of output
                tiles [128, 512] (dup=True) or [64, 512] (dup=False)."""
                pst = [
                    psP.tile([128, BLK], F32, tag="pp", name="pst") for _ in range(2)
                ]
                for c in range(NCH):
                    piece = xin.tile([128, 2 * BLK], F32, tag=f"in{tag}")
                    nc.sync.dma_start(
                        out=piece[:],
                        in_=x_ap[128 * c : 128 * c + 128, 1024 * sp : 1024 * sp + 1024],
                    )
                    for lo in range(2):
                        first, last = c == 0, c == NCH - 1
                        lw = w_sb[:, D_K * c : D_K * c + D_K]
                        rh = piece[:, BLK * lo : BLK * lo + BLK]
                        nc.tensor.matmul(
                            pst[lo][0:D_K, :], lw, rh, start=first, stop=last
                        )
                        if dup:
                            nc.tensor.matmul(
                                pst[lo][D_K:128, :],
                                lw,
                                rh,
                                start=first,
                                stop=last,
                                tile_position=(0, D_K),
                            )
                for lo in range(2):
                    b = 2 * sp + lo
                    if dup:
                        nc.scalar.copy(dst[b][:], pst[lo][:])
                    else:
                        nc.vector.tensor_copy(dst[b][:], pst[lo][0:D_K, :])

            def attention(bi):
                ps_o = psO.tile([65, BLK], F32, tag="po")
                njc = 4 * bi + 4
                for p in range(njc // 2):
                    jc0 = 2 * p
                    ps_s = psS.tile([128, 2 * BLK], F32, tag="ps")
                    for idx in range(2):
                        jc = jc0 + idx
                        sb, loc = jc // 4, jc % 4
                        ksrc = khT[sb][idx * D_K : idx * D_K + D_K,
                                       JC * loc : JC * loc + JC]
                        nc.tensor.matmul(
                            ps_s[:, BLK * idx : BLK * idx + BLK],
                            ksrc,
                            qhT[bi][idx * D_K : idx * D_K + D_K, :],
                            start=True,
                            stop=True,
                            tile_position=(idx * D_K, 0),
                        )
                    pt = pexp.tile([128, 2 * BLK], F32, tag="pt")
                    nc.scalar.activation(pt[:], ps_s[:], Exp, scale=0.125)
                    for idx in range(2):
                        jc = jc0 + idx
                        o = jc - 4 * bi
                        if o >= 0:
                            reg = slice(BLK * idx + JC * o, BLK * idx + JC * o + JC)
                            nc.vector.tensor_mul(pt[:, reg], pt[:, reg], tri_sb[:])
                        cs = JC * max(o, 0)
                        nc.tensor.matmul(
                            ps_o[:, cs:BLK],
                            vh_all[:, 65 * jc : 65 * jc + 65],
                            pt[:, BLK * idx + cs : BLK * idx + BLK],
                            start=(jc == 0),
                            stop=(jc == njc - 1),
                        )
                nc.vector.tensor_copy(OT[bi][:], ps_o[0:D_K, :])
                nc.vector.tensor_copy(rsrow[bi][:], ps_o[D_K : D_K + 1, :])

            # --- phase A+B interleaved: projections + attention ---
            for sp in range(NB // 2):
                proj(khT, wk_sb, kT, sp, "k", dup=True)
                proj(vhT, wv_sb, vT, sp, "v", dup=False)
                # transpose vhT -> vh_all chunks [j, dv] (+ ones col at 64)
                for lo in range(2):
                    b = 2 * sp + lo
                    for m in range(BLK // JC):
                        jc = b * (BLK // JC) + m
                        ps_t = psP.tile([128, D_K], F32, tag="pp")
                        nc.tensor.matmul(
                            ps_t[:],
                            vhT[b][:, JC * m : JC * m + JC],
                            id_sb[:],
                            is_transpose=True,
                            start=True,
                            stop=True,
                        )
                        nc.vector.tensor_copy(
                            vh_all[:, 65 * jc : 65 * jc + D_K], ps_t[:]
                        )
                proj(qhT, wq_sb, qT, sp, "q", dup=True)
                attention(2 * sp)
                attention(2 * sp + 1)

            # --- phase C: rowsum reciprocal + output projection ---
            ps_r = psS.tile([128, 4 * NB], F32, tag="ps")
            for bi in range(NB):
                for m in range(BLK // JC):
                    t = 4 * bi + m
                    nc.tensor.matmul(
                        ps_r[:, t : t + 1],
                        rsrow[bi][:, JC * m : JC * m + JC],
                        tri_sb[0:1, 0:1],
                        is_transpose=True,
                        start=(t == 0),
                        stop=(t == 4 * NB - 1),
                    )
            recip = persist.tile([128, 4 * NB], F32, tag="recip")
            nc.vector.reciprocal(recip[:], ps_r[:])

            for bi in range(NB):
                osb = osbp.tile([128, 4 * BLK], F32, tag="osb")
                for m in range(BLK // JC):
                    t = 4 * bi + m
                    ps_f = psS.tile([128, BLK], F32, tag="ps")
                    nc.tensor.matmul(
                        ps_f[:],
                        OT[bi][:, JC * m : JC * m + JC],
                        wf_sb[:],
                        start=True,
                        stop=True,
                    )
                    nc.vector.tensor_scalar_mul(
                        osb[:, BLK * m : BLK * m + BLK], ps_f[:], recip[:, t : t + 1]
                    )
                nc.sync.dma_start(
                    out=out[BLK * bi : BLK * bi + BLK, :].rearrange(
                        "(m p) e -> p m e", p=128
                    ),
                    in_=osb[:].rearrange("p (m e) -> p m e", m=4),
                )

    _split_multi_waits(nc)
    return nc


_CACHED = {}


def _get_nc(NB):
    if NB not in _CACHED:
        nc = bass.Bass(
            "TRN2", target_bir_lowering=False, debug=False, num_devices=N_CORES
        )
        _CACHED[NB] = build_kernel(nc, NB)
    return _CACHED[NB]


def kernel(q, k, v, Wq, Wk, Wv, Wf, bf, _trace=False):
    S = int(np.prod(q.shape[:-1]))
    NB = S // BLK
    nc = _get_nc(NB)

    qT = np.ascontiguousarray(q.reshape(S, DIM).T)
    kT = np.ascontiguousarray(k.reshape(S, DIM).T)
    vT = np.ascontiguousarray(v.reshape(S, DIM).T)
    tri = np.triu(np.ones((128, 128), dtype=np.float32))
    ident = np.eye(D_K, dtype=np.float32)

    in_maps = []
    for h in range(N_CORES):
        sl = slice(h * D_K, (h + 1) * D_K)
        in_maps.append(
            {
                "qT": qT,
                "kT": kT,
                "vT": vT,
                "wq": np.ascontiguousarray(np.asarray(Wq)[sl, :].T),
                "wk": np.ascontiguousarray(np.asarray(Wk)[sl, :].T),
                "wv": np.ascontiguousarray(np.asarray(Wv)[sl, :].T),
                "wf": np.ascontiguousarray(np.asarray(Wf)[:, sl].T),
                "tri": tri,
                "ident": ident,
            }
        )

    res = run_bass_kernel_spmd(nc, in_maps, list(range(N_CORES)), trace=_trace)
    total = None
    for h in range(N_CORES):
        part = res.results[h]["out"]
        total = part if total is None else total + part
    total = total + np.asarray(bf, dtype=np.float32)[None, :]
    outp = total.reshape(q.shape[:-1] + (DIM,)).astype(np.float32)
    if _trace:
        return outp, res
    return outp
